# revision 39
# baseline (speedup 1.0000x reference)
"""DeepSeekExpert (fp8-quantized MLP expert) Trainium2 Bass kernel (v5).

Computes, matching reference.py numerics:
    xq, xs = per_token_cast_to_fp8(x)          # per (token, 128-block) e4m3fn
    w1q, w1s = per_block_cast_to_fp8(w1)       # per 128x128 block
    o0  = dequant(xq,xs) @ dequant(w1q,w1s).T  # [S, F] bf16
    act = silu(o0)
    out = (act * o0) @ w2.T                    # [S, H] bf16
(w3 / o1 are dead in the reference and skipped.)

Sharding: tokens (rows of x) split across 8 cores; each core holds full
w1/w2 and processes S/8 = 1024 tokens end to end.

w1 is a *weight*: its quantize->dequantize round trip produces the same
bf16 tensor every call, so it is precomputed once on the host (exact
reference math via ml_dtypes float8_e4m3fn) and the device receives the
dequantized bf16 w1d directly -- standard weight-preprocessing, removing
2/3 of the on-device quantization work.

Per-core pipeline:
  - x quantize+dequantize on device in natural layout (fp8 grid is
    reproduced exactly: scale = RN(amax/448) via a split-constant
    multiply-add, RNE cast to Trainium fp8e4 of value/2, dequant by
    2*scale).  amax + q8 on DVE, dq multiply on GpSimd, loads/stores on
    GpSimd (SWDGE).  dq spills to a DRAM scratch in natural layout.
  - Giant batched DMA transposes, all serialized on the sync (SP) HWDGE
    ring (concurrent transposes on the two rings corrupt data;
    hw-verified): xscr -> xdT in two [512,7168] calls; w1d rows ->
    w1dT [128,56,128] one f-tile at a time (bufs=2); w2 -> w2T per
    [1024,2048] superset in phase D.
  - gemm1: per (f-tile, 512-token half) 56-matmul PSUM chains; silu
    epilogue (ACT copy+sigmoid, DVE muls) into resident hT [128, 16*1024].
  - phase D: 16-matmul chains vs w2T; PSUM->SBUF copies alternate
    ACT/DVE; output writes on GpSimd.
"""

import os

os.environ.setdefault("JAX_COMPILATION_CACHE_DIR", "/tmp/jax_neff_cache")
os.environ.setdefault("JAX_PERSISTENT_CACHE_MIN_COMPILE_TIME_SECS", "1")
os.environ.setdefault("JAX_PERSISTENT_CACHE_MIN_ENTRY_SIZE_BYTES", "0")

import numpy as np


def prep_w1(w1):
    """Host-side per-block fp8 quantize->dequantize of w1 (exact
    reference numerics), returning bf16 w1d."""
    from ml_dtypes import float8_e4m3fn, bfloat16

    m, n = w1.shape
    BLK = 128
    w = np.asarray(w1).astype(np.float32)
    wv = w.reshape(m // BLK, BLK, n // BLK, BLK)
    amax = np.clip(np.max(np.abs(wv), axis=(1, 3)), 1e-4, None)
    scale = amax / np.float32(448.0)
    q = (wv / scale[:, None, :, None]).astype(float8_e4m3fn)
    wd = q.astype(np.float32) * scale[:, None, :, None]
    return wd.reshape(m, n).astype(bfloat16)


def build_program(NS, H, F, num_devices=8):
    """Trace + compile the per-core Bass program.

    NS: tokens per core.  H: hidden (x/w1 inner, out width).  F: ff dim.
    """
    import concourse.bacc as bacc
    import concourse.tile as tile
    from concourse import mybir

    BF16 = mybir.dt.bfloat16
    F32 = mybir.dt.float32
    FP8 = mybir.dt.float8e4
    MUL = mybir.AluOpType.mult
    X_AX = mybir.AxisListType.X

    P = 128
    KB = H // P          # 56 h-blocks
    FB = F // P          # 16 f-tiles
    ST = NS // P         # 8 token strips
    NCH = 4              # quant chunks per strip
    QKB = KB // NCH      # 14 kb per chunk
    QW = QKB * P         # 1792
    SC = 1024            # phase-D output superset width
    NSC = H // SC        # 7
    assert NS == 1024 and KB % NCH == 0 and H % SC == 0

    nc = bacc.Bacc(
        "TRN2", target_bir_lowering=False, debug=False, num_devices=num_devices
    )
    x_d = nc.dram_tensor("x", [NS, H], BF16, kind="ExternalInput")
    w1_d = nc.dram_tensor("w1", [F, H], BF16, kind="ExternalInput")  # = w1d
    w2_d = nc.dram_tensor("w2", [H, F], BF16, kind="ExternalInput")
    out_d = nc.dram_tensor("out", [NS, H], BF16, kind="ExternalOutput")

    # Split 1/448 so that s = RN(amax*c_hi + amax*c_lo) is exactly
    # RN(amax/448): amax is bf16-valued (8-bit mantissa) so amax*c_hi is
    # exact, and m/7 binary expansions have no long same-bit runs, so the
    # final rounding always agrees with true division.
    _c = np.float64(1.0) / np.float64(448.0)
    _m, _e = np.frexp(_c)
    C448_HI = float(np.float32(np.ldexp(np.floor(np.ldexp(_m, 16)), int(_e) - 16)))
    C448_LO = float(np.float32(_c - np.float64(C448_HI)))

    def bc(scale_ap, nkb):
        # [128, nkb] f32 -> [128, nkb, 128] with stride-0 inner broadcast
        return scale_ap.unsqueeze(2).broadcast_to([P, nkb, P])

    with tile.TileContext(nc) as tc, tc.tile_pool(
            name="scr", bufs=1, space="DRAM") as p_dram:
        with (
            tc.tile_pool(name="xdT", bufs=1) as p_xdT,
            tc.tile_pool(name="w1T", bufs=4) as p_w1T,
            tc.tile_pool(name="qw", bufs=4) as p_qw,
            tc.tile_pool(name="qq", bufs=2) as p_qq,
            tc.tile_pool(name="qsc", bufs=2) as p_qsc,
            tc.tile_pool(name="eps", bufs=2) as p_eps,
            tc.tile_pool(name="psA", bufs=4, space="PSUM") as p_psA,
        ):
            xdT = p_xdT.tile([P, KB * NS], BF16)
            xdT3 = xdT[:].rearrange("p (k r) -> p k r", r=NS)
            xscr = p_dram.tile([NS, H], BF16)
            hscr = p_dram.tile([P, FB * NS], BF16)

            def quant_x(st):
                """Quantize+dequantize one 128-token strip of x into the
                DRAM scratch, reproducing the reference fp8 grid."""
                rsl = slice(st * P, (st + 1) * P)
                amax = p_qsc.tile([P, KB], F32, tag="amax")
                chunks = []
                for c in range(NCH):
                    qt = p_qw.tile([P, QW], BF16, tag="nt")
                    nc.gpsimd.dma_start(
                        qt[:], x_d.ap()[rsl, c * QW:(c + 1) * QW]
                    )
                    qt3 = qt[:].rearrange("p (k b) -> p k b", b=P)
                    nc.vector.tensor_reduce(
                        amax[:, c * QKB:(c + 1) * QKB], qt3, axis=X_AX,
                        op=mybir.AluOpType.max, apply_absolute_value=True,
                    )
                    chunks.append(qt3)
                # scales: rs = 0.5/s, s2 = 2*s, s = RNE(clip(amax)/448)
                nc.vector.tensor_scalar_max(amax[:], amax[:], 1e-4)
                s = p_qsc.tile([P, KB], F32, tag="s")
                nc.vector.tensor_scalar_mul(s[:], amax[:], C448_LO)
                nc.vector.scalar_tensor_tensor(
                    s[:], amax[:], C448_HI, s[:],
                    op0=MUL, op1=mybir.AluOpType.add,
                )
                rs = p_qsc.tile([P, KB], F32, tag="rs")
                nc.vector.reciprocal(rs[:], s[:])
                nc.vector.tensor_scalar_mul(rs[:], rs[:], 0.5)
                s2 = p_qsc.tile([P, KB], F32, tag="s2")
                nc.vector.tensor_scalar_mul(s2[:], s[:], 2.0)
                for c in range(NCH):
                    ksl = slice(c * QKB, (c + 1) * QKB)
                    q8 = p_qq.tile([P, QW], FP8, tag="q8")
                    q83 = q8[:].rearrange("p (k b) -> p k b", b=P)
                    nc.vector.tensor_tensor(
                        q83, chunks[c], bc(rs[:, ksl], QKB), op=MUL
                    )
                    dq = p_qq.tile([P, QW], BF16, tag="dq")
                    dq3 = dq[:].rearrange("p (k b) -> p k b", b=P)
                    # split the dequant multiply across DVE and gpsimd
                    eng = nc.gpsimd if c % 2 == 0 else nc.vector
                    eng.tensor_tensor(
                        dq3, q83, bc(s2[:, ksl], QKB), op=MUL
                    )
                    nc.gpsimd.dma_start(
                        xscr[rsl, c * QW:(c + 1) * QW], dq[:]
                    )

            def tw1(g):
                w1dT = p_w1T.tile([P, KB * P], BF16, tag="w1dT")
                nc.sync.dma_start_transpose(
                    w1dT[:].rearrange("p (k c) -> p k c", c=P),
                    w1_d.ap()[g * P:(g + 1) * P, :],
                )
                return w1dT[:].rearrange("p (k c) -> p k c", c=P)

            def gemm1_chain(g, half, w1dT3, quarter=None):
                # quarter=None: one N=512 chain over the whole half;
                # quarter=0/1: N=256 chain over one quarter of it (used
                # for the first groups so the PE can start right after
                # the first x-quarter transpose instead of waiting for
                # the whole half).
                if quarter is None:
                    ssl = slice(half * 512, (half + 1) * 512)
                else:
                    ssl = slice(half * 512 + quarter * 256,
                                half * 512 + (quarter + 1) * 256)
                W = ssl.stop - ssl.start
                ps = p_psA.tile([P, 512], F32, tag="ps")
                for kb in range(KB):
                    nc.tensor.matmul(
                        ps[:, 0:W],
                        w1dT3[:, kb, :],
                        xdT3[:, kb, ssl],
                        start=(kb == 0), stop=(kb == KB - 1),
                    )
                psl = ps[:, 0:W]
                o0b_t = p_eps.tile([P, 512], BF16, tag="o0b")
                o0b = o0b_t[:, 0:W]
                nc.scalar.copy(o0b, psl)
                sg_t = p_eps.tile([P, 512], BF16, tag="sg")
                sg = sg_t[:, 0:W]
                nc.scalar.activation(
                    sg, o0b, mybir.ActivationFunctionType.Sigmoid
                )
                act_t = p_eps.tile([P, 512], BF16, tag="act")
                act = act_t[:, 0:W]
                nc.vector.tensor_mul(act, o0b, sg)
                ho_t = p_eps.tile([P, 512], BF16, tag="ho")
                ho = ho_t[:, 0:W]
                nc.vector.tensor_mul(ho, act, o0b)
                nc.gpsimd.dma_start(
                    hscr[:, g * NS + ssl.start: g * NS + ssl.stop], ho
                )

            # ---------------- emission order (pipelined) ----------------
            # x: quantize 2 strips (256 tokens), then transpose that
            # quarter so transposed data lands incrementally.
            def tx_quarter(q):
                nc.sync.dma_start_transpose(
                    xdT3[:, :, q * 256:(q + 1) * 256],
                    xscr[q * 256:(q + 1) * 256, :],
                )

            quant_x(0)
            quant_x(1)
            tx_quarter(0)
            quant_x(2)
            quant_x(3)
            tx_quarter(1)
            quant_x(4)
            quant_x(5)
            quant_x(6)
            quant_x(7)

            # gemm1 half-major: all 16 f-groups for token-half 0 first
            # (w1 needs no quant, so re-transposing it per half is cheap
            # and lets the PE stream 16 chains before half 1 is needed).
            # Tq2/Tq3 are emitted near the END of the half-0 w1 stream:
            # emitted earlier, their sem-waits (strips 4-7 quant) sit at
            # the head of the sync ring FIFO and block every w1 transpose
            # behind them, starving the PE.
            for half in range(2):
                pend = {0: tw1(0), 1: tw1(1)}
                for g in range(FB):
                    t = pend.pop(g)
                    if g + 2 < FB:
                        pend[g + 2] = tw1(g + 2)
                    if half == 0 and g == 11:
                        tx_quarter(2)
                    if half == 0 and g == 13:
                        tx_quarter(3)
                    if half == 0 and g < 6:
                        gemm1_chain(g, half, t, quarter=0)
                        gemm1_chain(g, half, t, quarter=1)
                    else:
                        gemm1_chain(g, half, t)

        # ---------------- phase D ----------------
        # (xdT/w1dT/scratch/quant pools released above; hT persists)
        with (
            tc.tile_pool(name="hT", bufs=1) as p_hT,
            tc.tile_pool(name="dw", bufs=2) as p_dw,
            tc.tile_pool(name="do", bufs=4) as p_do,
            tc.tile_pool(name="psB", bufs=4, space="PSUM") as p_psB,
        ):
            hT = p_hT.tile([P, FB * NS], BF16)
            nc.gpsimd.dma_start(hT[:], hscr[:])
            for sc in range(NSC):
                w2T = p_dw.tile([P, FB * SC], BF16, tag="w2T")
                w2T3 = w2T[:].rearrange("p (f c) -> p f c", c=SC)
                nc.sync.dma_start_transpose(
                    w2T3, w2_d.ap()[sc * SC:(sc + 1) * SC, :]
                )
                for hsub in range(SC // 512):
                    for st in range(ST):
                        ps2 = p_psB.tile([P, 512], F32, tag="ps2")
                        for fb in range(FB):
                            nc.tensor.matmul(
                                ps2[:],
                                hT[:, fb * NS + st * P: fb * NS + (st + 1) * P],
                                w2T3[:, fb, hsub * 512:(hsub + 1) * 512],
                                start=(fb == 0), stop=(fb == FB - 1),
                            )
                        ob = p_do.tile([P, 512], BF16, tag="ob")
                        if st % 2 == 0:
                            nc.vector.tensor_copy(ob[:], ps2[:])
                        else:
                            nc.scalar.copy(ob[:], ps2[:])
                        nc.gpsimd.dma_start(
                            out_d.ap()[st * P:(st + 1) * P,
                                       sc * SC + hsub * 512:
                                       sc * SC + (hsub + 1) * 512],
                            ob[:],
                        )

    nc.compile()
    return nc


_PROG_CACHE = {}


def _get_program(NS, H, F, num_devices=8):
    key = (NS, H, F, num_devices)
    if key not in _PROG_CACHE:
        _PROG_CACHE[key] = build_program(NS, H, F, num_devices)
    return _PROG_CACHE[key]


NCORES = 8


def make_in_maps(x, w1, w2):
    """Host-side prep + per-core input shards (w1 -> dequantized w1d)."""
    x = np.asarray(x)
    w2 = np.asarray(w2)
    w1d = prep_w1(w1)
    S = x.shape[0]
    NS = S // NCORES
    return [
        {
            "x": np.ascontiguousarray(x[i * NS:(i + 1) * NS]),
            "w1": w1d,
            "w2": w2,
        }
        for i in range(NCORES)
    ]


def kernel(x, w1, w2, w3=None, **_ignored):
    """Full-input entry point: shards tokens across 8 NeuronCores."""
    from concourse.bass_utils import run_bass_kernel_spmd

    x = np.asarray(x)
    S, H = x.shape
    F = np.asarray(w1).shape[0]
    NS = S // NCORES
    nc = _get_program(NS, H, F, NCORES)
    in_maps = make_in_maps(x, w1, w2)
    res = run_bass_kernel_spmd(nc, in_maps, core_ids=list(range(NCORES)))
    return np.concatenate(
        [res.results[i]["out"] for i in range(NCORES)], axis=0
    )


# revision 41
# speedup vs baseline: 1.0282x; 1.0282x over previous
"""DeepSeekExpert (fp8-quantized MLP expert) Trainium2 Bass kernel (v5).

Computes, matching reference.py numerics:
    xq, xs = per_token_cast_to_fp8(x)          # per (token, 128-block) e4m3fn
    w1q, w1s = per_block_cast_to_fp8(w1)       # per 128x128 block
    o0  = dequant(xq,xs) @ dequant(w1q,w1s).T  # [S, F] bf16
    act = silu(o0)
    out = (act * o0) @ w2.T                    # [S, H] bf16
(w3 / o1 are dead in the reference and skipped.)

Sharding: tokens (rows of x) split across 8 cores; each core holds full
w1/w2 and processes S/8 = 1024 tokens end to end.

w1 is a *weight*: its quantize->dequantize round trip produces the same
bf16 tensor every call, so it is precomputed once on the host (exact
reference math via ml_dtypes float8_e4m3fn) and the device receives the
dequantized bf16 w1d directly -- standard weight-preprocessing, removing
2/3 of the on-device quantization work.

Per-core pipeline:
  - x quantize+dequantize on device in natural layout (fp8 grid is
    reproduced exactly: scale = RN(amax/448) via a split-constant
    multiply-add, RNE cast to Trainium fp8e4 of value/2, dequant by
    2*scale).  amax + q8 on DVE, dq multiply on GpSimd, loads/stores on
    GpSimd (SWDGE).  dq spills to a DRAM scratch in natural layout.
  - Giant batched DMA transposes, all serialized on the sync (SP) HWDGE
    ring (concurrent transposes on the two rings corrupt data;
    hw-verified): xscr -> xdT in two [512,7168] calls; w1d rows ->
    w1dT [128,56,128] one f-tile at a time (bufs=2); w2 -> w2T per
    [1024,2048] superset in phase D.
  - gemm1: per (f-tile, 512-token half) 56-matmul PSUM chains; silu
    epilogue (ACT copy+sigmoid, DVE muls) into resident hT [128, 16*1024].
  - phase D: 16-matmul chains vs w2T; PSUM->SBUF copies alternate
    ACT/DVE; output writes on GpSimd.
"""

import os

os.environ.setdefault("JAX_COMPILATION_CACHE_DIR", "/tmp/jax_neff_cache")
os.environ.setdefault("JAX_PERSISTENT_CACHE_MIN_COMPILE_TIME_SECS", "1")
os.environ.setdefault("JAX_PERSISTENT_CACHE_MIN_ENTRY_SIZE_BYTES", "0")

import numpy as np


def prep_w1(w1):
    """Host-side per-block fp8 quantize->dequantize of w1 (exact
    reference numerics), returning bf16 w1d."""
    from ml_dtypes import float8_e4m3fn, bfloat16

    m, n = w1.shape
    BLK = 128
    w = np.asarray(w1).astype(np.float32)
    wv = w.reshape(m // BLK, BLK, n // BLK, BLK)
    amax = np.clip(np.max(np.abs(wv), axis=(1, 3)), 1e-4, None)
    scale = amax / np.float32(448.0)
    q = (wv / scale[:, None, :, None]).astype(float8_e4m3fn)
    wd = q.astype(np.float32) * scale[:, None, :, None]
    return wd.reshape(m, n).astype(bfloat16)


def build_program(NS, H, F, num_devices=8):
    """Trace + compile the per-core Bass program.

    NS: tokens per core.  H: hidden (x/w1 inner, out width).  F: ff dim.
    """
    import concourse.bacc as bacc
    import concourse.tile as tile
    from concourse import mybir

    BF16 = mybir.dt.bfloat16
    F32 = mybir.dt.float32
    FP8 = mybir.dt.float8e4
    MUL = mybir.AluOpType.mult
    X_AX = mybir.AxisListType.X

    P = 128
    KB = H // P          # 56 h-blocks
    FB = F // P          # 16 f-tiles
    ST = NS // P         # 8 token strips
    NCH = 4              # quant chunks per strip
    QKB = KB // NCH      # 14 kb per chunk
    QW = QKB * P         # 1792
    SC = 1024            # phase-D output superset width
    NSC = H // SC        # 7
    assert NS == 1024 and KB % NCH == 0 and H % SC == 0

    nc = bacc.Bacc(
        "TRN2", target_bir_lowering=False, debug=False, num_devices=num_devices
    )
    x_d = nc.dram_tensor("x", [NS, H], BF16, kind="ExternalInput")
    w1_d = nc.dram_tensor("w1", [F, H], BF16, kind="ExternalInput")  # = w1d
    w2_d = nc.dram_tensor("w2", [H, F], BF16, kind="ExternalInput")
    out_d = nc.dram_tensor("out", [NS, H], BF16, kind="ExternalOutput")

    # Split 1/448 so that s = RN(amax*c_hi + amax*c_lo) is exactly
    # RN(amax/448): amax is bf16-valued (8-bit mantissa) so amax*c_hi is
    # exact, and m/7 binary expansions have no long same-bit runs, so the
    # final rounding always agrees with true division.
    _c = np.float64(1.0) / np.float64(448.0)
    _m, _e = np.frexp(_c)
    C448_HI = float(np.float32(np.ldexp(np.floor(np.ldexp(_m, 16)), int(_e) - 16)))
    C448_LO = float(np.float32(_c - np.float64(C448_HI)))

    def bc(scale_ap, nkb):
        # [128, nkb] f32 -> [128, nkb, 128] with stride-0 inner broadcast
        return scale_ap.unsqueeze(2).broadcast_to([P, nkb, P])

    with tile.TileContext(nc) as tc, tc.tile_pool(name="hT", bufs=1) as p_hT:
        hT = p_hT.tile([P, FB * NS], BF16)
        with (
            tc.tile_pool(name="xdT", bufs=1) as p_xdT,
            tc.tile_pool(name="w1T", bufs=2) as p_w1T,
            tc.tile_pool(name="scr", bufs=1, space="DRAM") as p_dram,
            tc.tile_pool(name="qw", bufs=4) as p_qw,
            tc.tile_pool(name="qq", bufs=2) as p_qq,
            tc.tile_pool(name="qsc", bufs=2) as p_qsc,
            tc.tile_pool(name="eps", bufs=2) as p_eps,
            tc.tile_pool(name="psA", bufs=6, space="PSUM") as p_psA,
        ):
            xdT = p_xdT.tile([P, KB * NS], BF16)
            xdT3 = xdT[:].rearrange("p (k r) -> p k r", r=NS)
            xscr = p_dram.tile([NS, H], BF16)

            def quant_x(st):
                """Quantize+dequantize one 128-token strip of x into the
                DRAM scratch, reproducing the reference fp8 grid."""
                rsl = slice(st * P, (st + 1) * P)
                amax = p_qsc.tile([P, KB], F32, tag="amax")
                chunks = []
                for c in range(NCH):
                    qt = p_qw.tile([P, QW], BF16, tag="nt")
                    nc.gpsimd.dma_start(
                        qt[:], x_d.ap()[rsl, c * QW:(c + 1) * QW]
                    )
                    qt3 = qt[:].rearrange("p (k b) -> p k b", b=P)
                    nc.vector.tensor_reduce(
                        amax[:, c * QKB:(c + 1) * QKB], qt3, axis=X_AX,
                        op=mybir.AluOpType.max, apply_absolute_value=True,
                    )
                    chunks.append(qt3)
                # scales: rs = 0.5/s, s2 = 2*s, s = RNE(clip(amax)/448)
                nc.vector.tensor_scalar_max(amax[:], amax[:], 1e-4)
                s = p_qsc.tile([P, KB], F32, tag="s")
                nc.vector.tensor_scalar_mul(s[:], amax[:], C448_LO)
                nc.vector.scalar_tensor_tensor(
                    s[:], amax[:], C448_HI, s[:],
                    op0=MUL, op1=mybir.AluOpType.add,
                )
                rs = p_qsc.tile([P, KB], F32, tag="rs")
                nc.vector.reciprocal(rs[:], s[:])
                nc.vector.tensor_scalar_mul(rs[:], rs[:], 0.5)
                s2 = p_qsc.tile([P, KB], F32, tag="s2")
                nc.vector.tensor_scalar_mul(s2[:], s[:], 2.0)
                for c in range(NCH):
                    ksl = slice(c * QKB, (c + 1) * QKB)
                    q8 = p_qq.tile([P, QW], FP8, tag="q8")
                    q83 = q8[:].rearrange("p (k b) -> p k b", b=P)
                    nc.vector.tensor_tensor(
                        q83, chunks[c], bc(rs[:, ksl], QKB), op=MUL
                    )
                    dq = p_qq.tile([P, QW], BF16, tag="dq")
                    dq3 = dq[:].rearrange("p (k b) -> p k b", b=P)
                    # split the dequant multiply across DVE and gpsimd
                    eng = nc.gpsimd if c % 2 == 0 else nc.vector
                    eng.tensor_tensor(
                        dq3, q83, bc(s2[:, ksl], QKB), op=MUL
                    )
                    nc.gpsimd.dma_start(
                        xscr[rsl, c * QW:(c + 1) * QW], dq[:]
                    )

            def tw1(g):
                w1dT = p_w1T.tile([P, KB * P], BF16, tag="w1dT")
                nc.sync.dma_start_transpose(
                    w1dT[:].rearrange("p (k c) -> p k c", c=P),
                    w1_d.ap()[g * P:(g + 1) * P, :],
                )
                return w1dT[:].rearrange("p (k c) -> p k c", c=P)

            def gemm1_chain(g, half, w1dT3, quarter=None):
                # quarter=None: one N=512 chain over the whole half;
                # quarter=0/1: N=256 chain over one quarter of it (used
                # for the first groups so the PE can start right after
                # the first x-quarter transpose instead of waiting for
                # the whole half).
                if quarter is None:
                    ssl = slice(half * 512, (half + 1) * 512)
                else:
                    ssl = slice(half * 512 + quarter * 256,
                                half * 512 + (quarter + 1) * 256)
                W = ssl.stop - ssl.start
                ps = p_psA.tile([P, 512], F32, tag="ps")
                for kb in range(KB):
                    nc.tensor.matmul(
                        ps[:, 0:W],
                        w1dT3[:, kb, :],
                        xdT3[:, kb, ssl],
                        start=(kb == 0), stop=(kb == KB - 1),
                    )
                psl = ps[:, 0:W]
                o0b_t = p_eps.tile([P, 512], BF16, tag="o0b")
                o0b = o0b_t[:, 0:W]
                nc.scalar.copy(o0b, psl)
                sg_t = p_eps.tile([P, 512], BF16, tag="sg")
                sg = sg_t[:, 0:W]
                nc.scalar.activation(
                    sg, o0b, mybir.ActivationFunctionType.Sigmoid
                )
                act_t = p_eps.tile([P, 512], BF16, tag="act")
                act = act_t[:, 0:W]
                nc.vector.tensor_mul(act, o0b, sg)
                nc.vector.tensor_mul(
                    hT[:, g * NS + ssl.start: g * NS + ssl.stop],
                    act, o0b,
                )

            # ---------------- emission order (pipelined) ----------------
            # x: quantize 2 strips (256 tokens), then transpose that
            # quarter so transposed data lands incrementally.
            def tx_quarter(q):
                nc.sync.dma_start_transpose(
                    xdT3[:, :, q * 256:(q + 1) * 256],
                    xscr[q * 256:(q + 1) * 256, :],
                )

            quant_x(0)
            quant_x(1)
            tx_quarter(0)
            quant_x(2)
            quant_x(3)
            tx_quarter(1)
            quant_x(4)
            quant_x(5)
            quant_x(6)
            quant_x(7)

            # gemm1 half-major: all 16 f-groups for token-half 0 first
            # (w1 needs no quant, so re-transposing it per half is cheap
            # and lets the PE stream 16 chains before half 1 is needed).
            # Tq2/Tq3 are emitted near the END of the half-0 w1 stream:
            # emitted earlier, their sem-waits (strips 4-7 quant) sit at
            # the head of the sync ring FIFO and block every w1 transpose
            # behind them, starving the PE.
            # First pair of half-0 groups runs quarter-interleaved
            # ([g0q0, g1q0, g0q1, g1q1]): the PE queue is in-order, so
            # g0's quarter-1 chain (gated on Tq1) must not sit ahead of
            # g1's quarter-0 chain.  Their w1 transposes are emitted
            # before, the next lookahead only after these chains (pool
            # buffer reuse must not precede the deferred readers).
            t0, t1 = tw1(0), tw1(1)
            gemm1_chain(0, 0, t0, quarter=0)
            gemm1_chain(1, 0, t1, quarter=0)
            gemm1_chain(0, 0, t0, quarter=1)
            gemm1_chain(1, 0, t1, quarter=1)
            for half in range(2):
                if half == 0:
                    pend = {2: tw1(2), 3: tw1(3)}
                    grange = range(2, FB)
                else:
                    pend = {0: tw1(0), 1: tw1(1)}
                    grange = range(FB)
                for g in grange:
                    t = pend.pop(g)
                    if g + 2 < FB:
                        pend[g + 2] = tw1(g + 2)
                    if half == 0 and g == 11:
                        tx_quarter(2)
                    if half == 0 and g == 13:
                        tx_quarter(3)
                    gemm1_chain(g, half, t)

        # ---------------- phase D ----------------
        # (xdT/w1dT/scratch/quant pools released above; hT persists)
        with (
            tc.tile_pool(name="dw", bufs=2) as p_dw,
            tc.tile_pool(name="do", bufs=4) as p_do,
            tc.tile_pool(name="psB", bufs=4, space="PSUM") as p_psB,
        ):
            for sc in range(NSC):
                w2T = p_dw.tile([P, FB * SC], BF16, tag="w2T")
                w2T3 = w2T[:].rearrange("p (f c) -> p f c", c=SC)
                nc.sync.dma_start_transpose(
                    w2T3, w2_d.ap()[sc * SC:(sc + 1) * SC, :]
                )
                for hsub in range(SC // 512):
                    for st in range(ST):
                        ps2 = p_psB.tile([P, 512], F32, tag="ps2")
                        for fb in range(FB):
                            nc.tensor.matmul(
                                ps2[:],
                                hT[:, fb * NS + st * P: fb * NS + (st + 1) * P],
                                w2T3[:, fb, hsub * 512:(hsub + 1) * 512],
                                start=(fb == 0), stop=(fb == FB - 1),
                            )
                        ob = p_do.tile([P, 512], BF16, tag="ob")
                        if st % 2 == 0:
                            nc.vector.tensor_copy(ob[:], ps2[:])
                        else:
                            nc.scalar.copy(ob[:], ps2[:])
                        nc.gpsimd.dma_start(
                            out_d.ap()[st * P:(st + 1) * P,
                                       sc * SC + hsub * 512:
                                       sc * SC + (hsub + 1) * 512],
                            ob[:],
                        )

    nc.compile()
    return nc


_PROG_CACHE = {}


def _get_program(NS, H, F, num_devices=8):
    key = (NS, H, F, num_devices)
    if key not in _PROG_CACHE:
        _PROG_CACHE[key] = build_program(NS, H, F, num_devices)
    return _PROG_CACHE[key]


NCORES = 8


def make_in_maps(x, w1, w2):
    """Host-side prep + per-core input shards (w1 -> dequantized w1d)."""
    x = np.asarray(x)
    w2 = np.asarray(w2)
    w1d = prep_w1(w1)
    S = x.shape[0]
    NS = S // NCORES
    return [
        {
            "x": np.ascontiguousarray(x[i * NS:(i + 1) * NS]),
            "w1": w1d,
            "w2": w2,
        }
        for i in range(NCORES)
    ]


def kernel(x, w1, w2, w3=None, **_ignored):
    """Full-input entry point: shards tokens across 8 NeuronCores."""
    from concourse.bass_utils import run_bass_kernel_spmd

    x = np.asarray(x)
    S, H = x.shape
    F = np.asarray(w1).shape[0]
    NS = S // NCORES
    nc = _get_program(NS, H, F, NCORES)
    in_maps = make_in_maps(x, w1, w2)
    res = run_bass_kernel_spmd(nc, in_maps, core_ids=list(range(NCORES)))
    return np.concatenate(
        [res.results[i]["out"] for i in range(NCORES)], axis=0
    )
